# revision 12
# baseline (speedup 1.0000x reference)
"""Pointer-style attention kernel for Trainium2, SPMD over 8 NeuronCores.

Reference computation (per full batch B=128, S=2048, E=H=512):
    q  = query @ Wq.T + bq                    [B, H]
    k  = target @ Wk.T + bk                   [B, S, H]
    qk = einsum('bh,bsh->bs', q, k)           [B, S]
    qk = 10 * tanh(qk);  qk[mask==1] = -inf
    alpha = softmax(qk, axis=-1)

Key algebraic reformulation (exact in exact arithmetic):
    qk[b,s] = target[b,s,:] . qp[b,:] + qb[b]
      qp = query @ (Wq.T @ Wk) + bq @ Wk      [B, E]
      qb = query @ (Wq.T @ bk) + bq . bk      [B]
This collapses the S*E*H einsum (137 GFLOP) into an S*E dot-product
stream (0.27 GFLOP), making the kernel HBM-bound on streaming `target`.

Distribution: data-parallel over batch; 16 batches per core, weights
replicated, no cross-core communication (softmax is per-row).

Per-core plan:
  - tiny preamble on TensorE builds M=Wq.T@Wk, qp, qb and per-batch
    broadcasts of qp/qb across 128 partitions (ones-vector matmuls)
  - main loop streams target[b] (4 MB per batch) via HWDGE DMA into
    [128, 16, 512] tiles; VectorE fused tensor_tensor_reduce computes
    scores[s-chunk] = qb + sum_e target[s,e]*qp[e] per 128-row chunk
  - epilogue: tanh/exp on ScalarE, masking, per-row softmax in an
    s-on-partitions layout (cross-partition sums via ones matmuls),
    TensorE transpose back to row-major, contiguous DMA out.
"""

import sys
import types

import numpy as np

B, S, E, H = 128, 2048, 512, 512
NCORES = 8
BS = B // NCORES  # 16 batches per core
KC = S // 128  # 16 s-chunks of 128 per batch
EC = E // 128  # 4 e-chunks of 128


def _install_axon_profile_shim():
    """Make run_bass_kernel_spmd(trace=True) usable in this container:
    provide antenv.axon_hooks (NTFF profile hook via ctypes into the
    axon PJRT .so) and stub the S3 artifact upload."""
    try:
        if "antenv.axon_hooks" not in sys.modules:
            import antenv
            from trn_agent_boot.trn_boot import _ntff_profile_via_ctypes

            hook = _ntff_profile_via_ctypes("/opt/axon/libaxon_pjrt.so")
            mod = types.ModuleType("antenv.axon_hooks")
            mod._hook = hook
            mod.get_axon_ntff_profile_hook = lambda: mod._hook

            def _set(h):
                mod._hook = h

            mod.set_axon_ntff_profile_hook = _set
            sys.modules["antenv.axon_hooks"] = mod
            antenv.axon_hooks = mod
    except Exception:
        pass
    try:
        import concourse.bass_utils as bu

        bu.upload_artifacts = lambda tmpdir: str(tmpdir)
    except Exception:
        pass


def _legalize_sync_waits(nc):
    """This walrus build rejects instructions carrying more than a couple
    of sync-wait commands. After Tile scheduling, split each instruction's
    excess waits onto same-engine NOPs inserted immediately before it —
    sequencers execute in order, so semantics are identical."""
    import bass_rust
    from concourse import mybir

    n_split = 0
    for f in nc.m.functions:
        for blk in f.blocks:
            il = blk.instructions
            out = []
            changed = False
            for inst in il:
                si = inst.sync_info
                waits = list(si.on_wait) if si is not None else []
                cap = 2 if isinstance(inst, mybir.InstEventSemaphore) else 1
                if len(waits) > cap:
                    rest = waits[: len(waits) - cap]
                    for j, w in enumerate(rest):
                        nop = mybir.InstNoOp(
                            name=f"{inst.name}-swait{j}",
                            engine=inst.engine,
                            bass_nofuse=True,
                            sync_info=bass_rust.SyncInfo(on_wait=[w], on_update=[]),
                        )
                        out.append(nop)
                        n_split += 1
                    si.on_wait = waits[len(waits) - cap :]
                    inst.sync_info = si
                    changed = True
                out.append(inst)
            if changed:
                blk.instructions = out
    return n_split


def build_kernel():
    import concourse.bass as bass
    from concourse import mybir
    from concourse.masks import make_identity

    import concourse.tile as tile

    f32 = mybir.dt.float32
    i32 = mybir.dt.int32
    Alu = mybir.AluOpType
    Act = mybir.ActivationFunctionType

    TC = tile.TileContext

    nc = bass.Bass()
    query_d = nc.dram_tensor("query", [BS, E], f32, kind="ExternalInput")
    target_d = nc.dram_tensor("target", [BS, S, E], f32, kind="ExternalInput")
    mask_d = nc.dram_tensor("mask", [BS, S], i32, kind="ExternalInput")
    wq_d = nc.dram_tensor("Wq", [H, E], f32, kind="ExternalInput")
    bq_d = nc.dram_tensor("bq", [H], f32, kind="ExternalInput")
    wk_d = nc.dram_tensor("Wk", [H, E], f32, kind="ExternalInput")
    bk_d = nc.dram_tensor("bk", [H], f32, kind="ExternalInput")
    alpha_d = nc.dram_tensor("alpha", [BS, S], f32, kind="ExternalOutput")

    with TC(nc) as tc:
        with (
            tc.tile_pool(name="singles", bufs=1) as singles,
            tc.tile_pool(name="tgt", bufs=2) as tgtp,
            tc.tile_pool(name="trash", bufs=2) as trashp,
            tc.tile_pool(name="prod", bufs=2) as prodp,
            tc.tile_pool(name="ppre", bufs=2, space="PSUM") as ppre,
            tc.tile_pool(name="pqpb", bufs=2, space="PSUM") as pqpb,
            tc.tile_pool(name="pepi", bufs=2, space="PSUM") as pepi,
        ):
            # ---- constant / weight loads -------------------------------
            wq_sb = singles.tile([128, EC, E], f32)  # [p, h-chunk, e']
            wk_sb = singles.tile([128, EC, E], f32)
            nc.sync.dma_start(out=wq_sb, in_=wq_d.rearrange("(c p) e -> p c e", p=128))
            nc.sync.dma_start(out=wk_sb, in_=wk_d.rearrange("(c p) e -> p c e", p=128))
            query_sb = singles.tile([BS, E], f32)
            nc.sync.dma_start(out=query_sb, in_=query_d[:, :])
            bq_row = singles.tile([1, H], f32)
            bk_row = singles.tile([1, H], f32)
            nc.sync.dma_start(out=bq_row, in_=bq_d[None, :])
            nc.sync.dma_start(out=bk_row, in_=bk_d[None, :])
            mask_sb = singles.tile([BS, S], i32)
            nc.sync.dma_start(out=mask_sb, in_=mask_d[:, :])

            ident = singles.tile([128, 128], f32)
            make_identity(nc, ident)
            ones_row = singles.tile([1, 128], f32)  # lhsT for partition-bcast
            nc.vector.memset(ones_row, 1.0)
            ones_col = singles.tile([128, 1], f32)  # rhs/lhsT for partition-sum
            nc.vector.memset(ones_col, 1.0)
            # Row-selector weights: sel[:, b*128:(b+1)*128].T @ X broadcasts
            # X's row b across all 128 output partitions (PE operands must
            # start at partition 0, so we can't slice X[b:b+1] directly).
            sel_sb = singles.tile([BS, BS, 128], f32)
            nc.gpsimd.memset(sel_sb, 0.0)
            # sel[j, b, p] = (j == b): iota = j*1 + b*(-1) + p*0; fill where ==0
            nc.gpsimd.affine_select(
                out=sel_sb,
                in_=sel_sb,
                compare_op=Alu.not_equal,
                fill=1.0,
                base=0,
                pattern=[[-1, BS], [0, 128]],
                channel_multiplier=1,
            )

            # ---- transpose bq, bk into [128, EC] column form ------------
            bqT = singles.tile([128, EC], f32)
            bkT = singles.tile([128, EC], f32)
            for c in range(EC):
                pt = ppre.tile([128, 1], f32, tag="pre")
                nc.tensor.transpose(pt, bq_row[0:1, c * 128 : (c + 1) * 128], ident[0:1, 0:1])
                nc.vector.tensor_copy(bqT[:, c : c + 1], pt)
                pt2 = ppre.tile([128, 1], f32, tag="pre")
                nc.tensor.transpose(pt2, bk_row[0:1, c * 128 : (c + 1) * 128], ident[0:1, 0:1])
                nc.vector.tensor_copy(bkT[:, c : c + 1], pt2)

            # ---- M = Wq.T @ Wk  [e', e] --------------------------------
            m_sb = singles.tile([128, EC, E], f32)  # [p, e'-chunk, e]
            for m in range(EC):
                pm = ppre.tile([128, E], f32, tag="pre")
                for c in range(EC):
                    nc.tensor.matmul(
                        pm,
                        wq_sb[:, c, m * 128 : (m + 1) * 128],
                        wk_sb[:, c, :],
                        start=(c == 0),
                        stop=(c == EC - 1),
                    )
                nc.scalar.copy(m_sb[:, m, :], pm)

            # ---- queryT [e', b] ----------------------------------------
            qT_sb = singles.tile([128, EC, BS], f32)
            for m in range(EC):
                pq = ppre.tile([128, BS], f32, tag="pre")
                nc.tensor.transpose(pq, query_sb[:, m * 128 : (m + 1) * 128], ident[0:BS, 0:BS])
                nc.vector.tensor_copy(qT_sb[:, m, :], pq)

            # ---- qbias[e] = bq @ Wk  (row [1, E]) ----------------------
            qbias_sb = singles.tile([1, E], f32)
            pqb = ppre.tile([1, E], f32, tag="pre")
            for c in range(EC):
                nc.tensor.matmul(
                    pqb, bqT[:, c : c + 1], wk_sb[:, c, :],
                    start=(c == 0), stop=(c == EC - 1),
                )
            nc.scalar.copy(qbias_sb, pqb)

            # ---- qp[b, e] = query @ M  [BS, E] -------------------------
            qp_sb = singles.tile([BS, E], f32)
            pqp = ppre.tile([BS, E], f32, tag="pre")
            for m in range(EC):
                nc.tensor.matmul(
                    pqp, qT_sb[:, m, :], m_sb[:, m, :],
                    start=(m == 0), stop=(m == EC - 1),
                )
            nc.scalar.copy(qp_sb, pqp)

            # ---- v[e'] = Wq.T @ bk;  qb_raw[b] = query @ v -------------
            v_sb = singles.tile([128, EC], f32)
            for m in range(EC):
                pv = ppre.tile([128, 1], f32, tag="pre")
                for c in range(EC):
                    nc.tensor.matmul(
                        pv, wq_sb[:, c, m * 128 : (m + 1) * 128], bkT[:, c : c + 1],
                        start=(c == 0), stop=(c == EC - 1),
                    )
                nc.vector.tensor_copy(v_sb[:, m : m + 1], pv)
            qb_sb = singles.tile([BS, 1], f32)
            pqbv = ppre.tile([BS, 1], f32, tag="pre")
            for m in range(EC):
                nc.tensor.matmul(
                    pqbv, qT_sb[:, m, :], v_sb[:, m : m + 1],
                    start=(m == 0), stop=(m == EC - 1),
                )
            nc.vector.tensor_copy(qb_sb, pqbv)

            # ---- dot(bq, bk) scalar ------------------------------------
            trash4 = singles.tile([128, EC], f32)
            dotp = singles.tile([128, 1], f32)
            nc.vector.tensor_mul(trash4, bqT, bkT)
            nc.vector.tensor_reduce(
                dotp, trash4, axis=mybir.AxisListType.X, op=Alu.add
            )
            pdot = ppre.tile([1, 1], f32, tag="pre")
            nc.tensor.matmul(pdot, dotp, ones_col, start=True, stop=True)
            dot_sb = singles.tile([1, 1], f32)
            nc.vector.tensor_copy(dot_sb, pdot)

            # ---- qb row [1, BS] = qb_raw.T + dot(bq,bk) ----------------
            pqbrow = ppre.tile([1, BS], f32, tag="pre")
            nc.tensor.transpose(pqbrow, qb_sb, ident[0:BS, 0:BS])
            qbrow_sb = singles.tile([1, BS], f32)
            nc.scalar.activation(
                qbrow_sb, pqbrow, Act.Identity, bias=dot_sb[0:1, 0:1], scale=1.0
            )
            # broadcast to all partitions: qbb[p, b] = qb[b]
            pqbb = ppre.tile([128, BS], f32, tag="pre")
            nc.tensor.matmul(pqbb, ones_row, qbrow_sb, start=True, stop=True)
            qbb_sb = singles.tile([128, BS], f32)
            nc.vector.tensor_copy(qbb_sb, pqbb)

            # ---- mask -> m01T [p, b, k]  (1.0 = keep, 0.0 = masked) ----
            maskf = singles.tile([BS, S], f32)
            nc.vector.tensor_copy(maskf, mask_sb)  # int32 -> f32 convert
            m01 = singles.tile([BS, S], f32)
            nc.vector.tensor_scalar(
                out=m01, in0=maskf, scalar1=-1.0, scalar2=1.0,
                op0=Alu.mult, op1=Alu.add,
            )
            m01T = singles.tile([128, BS, KC], f32)
            for k in range(KC):
                pmt = pepi.tile([128, BS], f32, tag="epi")
                nc.tensor.transpose(pmt, m01[:, k * 128 : (k + 1) * 128], ident[0:BS, 0:BS])
                nc.vector.tensor_copy(m01T[:, :, k], pmt)

            # ---- main loop: scores[p, b, k] ----------------------------
            # One giant [128, KC*E] multiply per batch amortizes the DVE
            # per-instruction overhead; the e-reduction is split between
            # VectorE (giant innermost-dim tensor_reduce, 5 batches) and
            # ScalarE (per-chunk activation accum, 11 batches) to balance
            # both engines just under the HBM-stream floor.
            scores = singles.tile([128, BS, KC], f32)
            dve_red = {2, 5, 8, 11, 14}
            for b in range(BS):
                tgt = tgtp.tile([128, KC, E], f32, tag="tgt")
                nc.sync.dma_start(
                    out=tgt, in_=target_d[b].rearrange("(k p) e -> p k e", p=128)
                )
                # qp broadcast for this batch (+ qbias), all 128 partitions
                pb = pqpb.tile([128, E], f32, tag="qpb")
                nc.tensor.matmul(
                    pb, sel_sb[:, b, :], qp_sb, start=True, stop=False
                )
                nc.tensor.matmul(pb, ones_row, qbias_sb, start=False, stop=True)
                prod = prodp.tile([128, KC, E], f32, tag="prod")
                pb_b = bass.AP(
                    tensor=pb.tensor, offset=pb.offset,
                    ap=[pb.ap[0], [0, KC], pb.ap[1]],
                )
                nc.vector.tensor_mul(prod, tgt, pb_b)
                if b in dve_red:
                    nc.vector.tensor_reduce(
                        scores[:, b, :], prod, axis=mybir.AxisListType.X, op=Alu.add
                    )
                else:
                    for k in range(KC):
                        tr = trashp.tile([128, E], mybir.dt.bfloat16, tag="trash")
                        nc.scalar.activation(
                            tr, prod[:, k, :], Act.Copy,
                            accum_out=scores[:, b, k : k + 1],
                        )

            # ---- softmax epilogue (s on partitions) --------------------
            # scores += qb[b]  (ACT accum_out cannot carry an initial value)
            scores2 = singles.tile([128, BS, KC], f32)
            for b in range(BS):
                nc.vector.tensor_scalar(
                    out=scores2[:, b, :], in0=scores[:, b, :],
                    scalar1=qbb_sb[:, b : b + 1], scalar2=None, op0=Alu.add,
                )
            t_sb = singles.tile([128, BS, KC], f32)
            nc.scalar.activation(t_sb, scores2, Act.Tanh)
            e_sb = singles.tile([128, BS, KC], f32)
            nc.scalar.activation(e_sb, t_sb, Act.Exp, scale=10.0)
            e2_sb = singles.tile([128, BS, KC], f32)
            nc.vector.tensor_mul(e2_sb, e_sb, m01T)

            part_sb = singles.tile([128, BS], f32)
            for b in range(BS):
                nc.vector.tensor_reduce(
                    part_sb[:, b : b + 1], e2_sb[:, b, :],
                    axis=mybir.AxisListType.X, op=Alu.add,
                )
            pden = pepi.tile([1, BS], f32, tag="epi")
            nc.tensor.matmul(pden, ones_col, part_sb, start=True, stop=True)
            recip_sb = singles.tile([1, BS], f32)
            nc.vector.reciprocal(recip_sb, pden)
            prb = pepi.tile([128, BS], f32, tag="epi")
            nc.tensor.matmul(prb, ones_row, recip_sb, start=True, stop=True)
            rb_sb = singles.tile([128, BS], f32)
            nc.vector.tensor_copy(rb_sb, prb)

            a_sb = singles.tile([128, BS, KC], f32)
            for b in range(BS):
                nc.vector.tensor_scalar(
                    out=a_sb[:, b, :], in0=e2_sb[:, b, :],
                    scalar1=rb_sb[:, b : b + 1], scalar2=None, op0=Alu.mult,
                )

            # ---- transpose to row-major [b*KC + k, p] and store --------
            alpha_flat = alpha_d.rearrange("b (k p) -> (b k) p", p=128)
            for h in range(2):
                pat = pepi.tile([128, 128], f32, tag="epi")
                nc.tensor.transpose(
                    pat, a_sb[:, h * (BS // 2) : (h + 1) * (BS // 2), :], ident
                )
                at_sb = singles.tile([128, 128], f32, tag=f"at{h}")
                nc.vector.tensor_copy(at_sb, pat)
                nc.sync.dma_start(
                    out=alpha_flat[h * 128 : (h + 1) * 128, :], in_=at_sb
                )

    _legalize_sync_waits(nc)
    return nc


_NC_CACHE = None


def kernel(query, target, mask, Wq, bq, Wk, bk):
    global _NC_CACHE
    _install_axon_profile_shim()
    from concourse.bass_utils import run_bass_kernel_spmd

    query = np.ascontiguousarray(np.asarray(query, dtype=np.float32))
    target = np.ascontiguousarray(np.asarray(target, dtype=np.float32))
    mask = np.ascontiguousarray(np.asarray(mask, dtype=np.int32))
    Wq = np.ascontiguousarray(np.asarray(Wq, dtype=np.float32))
    bq = np.ascontiguousarray(np.asarray(bq, dtype=np.float32))
    Wk = np.ascontiguousarray(np.asarray(Wk, dtype=np.float32))
    bk = np.ascontiguousarray(np.asarray(bk, dtype=np.float32))

    if _NC_CACHE is None:
        _NC_CACHE = build_kernel()
    nc = _NC_CACHE

    in_maps = []
    for i in range(NCORES):
        sl = slice(i * BS, (i + 1) * BS)
        in_maps.append(
            {
                "query": query[sl],
                "target": target[sl],
                "mask": mask[sl],
                "Wq": Wq,
                "bq": bq,
                "Wk": Wk,
                "bk": bk,
            }
        )

    res = run_bass_kernel_spmd(nc, in_maps, list(range(NCORES)))
    out = np.concatenate([res.results[i]["alpha"] for i in range(NCORES)], axis=0)
    return out.astype(np.float32)


# revision 13
# speedup vs baseline: 1.1397x; 1.1397x over previous
"""Pointer-style attention kernel for Trainium2, SPMD over 8 NeuronCores.

Reference computation (per full batch B=128, S=2048, E=H=512):
    q  = query @ Wq.T + bq                    [B, H]
    k  = target @ Wk.T + bk                   [B, S, H]
    qk = einsum('bh,bsh->bs', q, k)           [B, S]
    qk = 10 * tanh(qk);  qk[mask==1] = -inf
    alpha = softmax(qk, axis=-1)

Key algebraic reformulation (exact in exact arithmetic):
    qk[b,s] = target[b,s,:] . qp[b,:] + qb[b]
      qp = query @ (Wq.T @ Wk) + bq @ Wk      [B, E]
      qb = query @ (Wq.T @ bk) + bq . bk      [B]
This collapses the S*E*H einsum (137 GFLOP) into an S*E dot-product
stream (0.27 GFLOP), making the kernel HBM-bound on streaming `target`.

Distribution: data-parallel over batch; 16 batches per core, weights
replicated, no cross-core communication (softmax is per-row).

Per-core plan:
  - tiny preamble on TensorE builds M=Wq.T@Wk, qp, qb and per-batch
    broadcasts of qp/qb across 128 partitions (ones-vector matmuls)
  - main loop streams target[b] (4 MB per batch) via HWDGE DMA into
    [128, 16, 512] tiles; VectorE fused tensor_tensor_reduce computes
    scores[s-chunk] = qb + sum_e target[s,e]*qp[e] per 128-row chunk
  - epilogue: tanh/exp on ScalarE, masking, per-row softmax in an
    s-on-partitions layout (cross-partition sums via ones matmuls),
    TensorE transpose back to row-major, contiguous DMA out.
"""

import sys
import types

import numpy as np

B, S, E, H = 128, 2048, 512, 512
NCORES = 8
BS = B // NCORES  # 16 batches per core
KC = S // 128  # 16 s-chunks of 128 per batch
EC = E // 128  # 4 e-chunks of 128


def _install_axon_profile_shim():
    """Make run_bass_kernel_spmd(trace=True) usable in this container:
    provide antenv.axon_hooks (NTFF profile hook via ctypes into the
    axon PJRT .so) and stub the S3 artifact upload."""
    try:
        if "antenv.axon_hooks" not in sys.modules:
            import antenv
            from trn_agent_boot.trn_boot import _ntff_profile_via_ctypes

            hook = _ntff_profile_via_ctypes("/opt/axon/libaxon_pjrt.so")
            mod = types.ModuleType("antenv.axon_hooks")
            mod._hook = hook
            mod.get_axon_ntff_profile_hook = lambda: mod._hook

            def _set(h):
                mod._hook = h

            mod.set_axon_ntff_profile_hook = _set
            sys.modules["antenv.axon_hooks"] = mod
            antenv.axon_hooks = mod
    except Exception:
        pass
    try:
        import concourse.bass_utils as bu

        bu.upload_artifacts = lambda tmpdir: str(tmpdir)
    except Exception:
        pass


def _legalize_sync_waits(nc):
    """This walrus build rejects instructions carrying more than a couple
    of sync-wait commands. After Tile scheduling, split each instruction's
    excess waits onto same-engine NOPs inserted immediately before it —
    sequencers execute in order, so semantics are identical."""
    import bass_rust
    from concourse import mybir

    n_split = 0
    for f in nc.m.functions:
        for blk in f.blocks:
            il = blk.instructions
            out = []
            changed = False
            for inst in il:
                si = inst.sync_info
                waits = list(si.on_wait) if si is not None else []
                cap = 2 if isinstance(inst, mybir.InstEventSemaphore) else 1
                if len(waits) > cap:
                    rest = waits[: len(waits) - cap]
                    for j, w in enumerate(rest):
                        nop = mybir.InstNoOp(
                            name=f"{inst.name}-swait{j}",
                            engine=inst.engine,
                            bass_nofuse=True,
                            sync_info=bass_rust.SyncInfo(on_wait=[w], on_update=[]),
                        )
                        out.append(nop)
                        n_split += 1
                    si.on_wait = waits[len(waits) - cap :]
                    inst.sync_info = si
                    changed = True
                out.append(inst)
            if changed:
                blk.instructions = out
    return n_split


def build_kernel():
    import concourse.bass as bass
    from concourse import mybir
    from concourse.masks import make_identity

    import concourse.tile as tile

    f32 = mybir.dt.float32
    i32 = mybir.dt.int32
    Alu = mybir.AluOpType
    Act = mybir.ActivationFunctionType

    TC = tile.TileContext

    nc = bass.Bass()
    query_d = nc.dram_tensor("query", [BS, E], f32, kind="ExternalInput")
    target_d = nc.dram_tensor("target", [BS, S, E], f32, kind="ExternalInput")
    mask_d = nc.dram_tensor("mask", [BS, S], i32, kind="ExternalInput")
    wq_d = nc.dram_tensor("Wq", [H, E], f32, kind="ExternalInput")
    bq_d = nc.dram_tensor("bq", [H], f32, kind="ExternalInput")
    wk_d = nc.dram_tensor("Wk", [H, E], f32, kind="ExternalInput")
    bk_d = nc.dram_tensor("bk", [H], f32, kind="ExternalInput")
    alpha_d = nc.dram_tensor("alpha", [BS, S], f32, kind="ExternalOutput")

    with TC(nc) as tc:
        with (
            tc.tile_pool(name="singles", bufs=1) as singles,
            tc.tile_pool(name="tgt", bufs=5) as tgtp,
            tc.tile_pool(name="trash", bufs=2) as trashp,
            tc.tile_pool(name="prod", bufs=3) as prodp,
            tc.tile_pool(name="ppre", bufs=2, space="PSUM") as ppre,
            tc.tile_pool(name="pqpb", bufs=3, space="PSUM") as pqpb,
            tc.tile_pool(name="pepi", bufs=2, space="PSUM") as pepi,
        ):
            # ---- constant / weight loads -------------------------------
            wq_sb = singles.tile([128, EC, E], f32)  # [p, h-chunk, e']
            wk_sb = singles.tile([128, EC, E], f32)
            for c in range(EC):
                nc.sync.dma_start(out=wq_sb[:, c, :], in_=wq_d[c * 128 : (c + 1) * 128, :])
                nc.sync.dma_start(out=wk_sb[:, c, :], in_=wk_d[c * 128 : (c + 1) * 128, :])
            query_sb = singles.tile([BS, E], f32)
            nc.sync.dma_start(out=query_sb, in_=query_d[:, :])
            bq_row = singles.tile([1, H], f32)
            bk_row = singles.tile([1, H], f32)
            nc.sync.dma_start(out=bq_row, in_=bq_d[None, :])
            nc.sync.dma_start(out=bk_row, in_=bk_d[None, :])
            mask_sb = singles.tile([BS, S], i32)
            nc.sync.dma_start(out=mask_sb, in_=mask_d[:, :])

            ident = singles.tile([128, 128], f32)
            make_identity(nc, ident)
            ones_row = singles.tile([1, 128], f32)  # lhsT for partition-bcast
            nc.vector.memset(ones_row, 1.0)
            ones_col = singles.tile([128, 1], f32)  # rhs/lhsT for partition-sum
            nc.vector.memset(ones_col, 1.0)
            # Row-selector weights: sel[:, b*128:(b+1)*128].T @ X broadcasts
            # X's row b across all 128 output partitions (PE operands must
            # start at partition 0, so we can't slice X[b:b+1] directly).
            sel_sb = singles.tile([BS, BS, 128], f32)
            nc.gpsimd.memset(sel_sb, 0.0)
            # sel[j, b, p] = (j == b): iota = j*1 + b*(-1) + p*0; fill where ==0
            nc.gpsimd.affine_select(
                out=sel_sb,
                in_=sel_sb,
                compare_op=Alu.not_equal,
                fill=1.0,
                base=0,
                pattern=[[-1, BS], [0, 128]],
                channel_multiplier=1,
            )

            # ---- transpose bq, bk into [128, EC] column form ------------
            bqT = singles.tile([128, EC], f32)
            bkT = singles.tile([128, EC], f32)
            for c in range(EC):
                pt = ppre.tile([128, 1], f32, tag="pre")
                nc.tensor.transpose(pt, bq_row[0:1, c * 128 : (c + 1) * 128], ident[0:1, 0:1])
                nc.vector.tensor_copy(bqT[:, c : c + 1], pt)
                pt2 = ppre.tile([128, 1], f32, tag="pre")
                nc.tensor.transpose(pt2, bk_row[0:1, c * 128 : (c + 1) * 128], ident[0:1, 0:1])
                nc.vector.tensor_copy(bkT[:, c : c + 1], pt2)

            # ---- M = Wq.T @ Wk  [e', e] --------------------------------
            m_sb = singles.tile([128, EC, E], f32)  # [p, e'-chunk, e]
            for m in range(EC):
                pm = ppre.tile([128, E], f32, tag="pre")
                for c in range(EC):
                    nc.tensor.matmul(
                        pm,
                        wq_sb[:, c, m * 128 : (m + 1) * 128],
                        wk_sb[:, c, :],
                        start=(c == 0),
                        stop=(c == EC - 1),
                    )
                nc.scalar.copy(m_sb[:, m, :], pm)

            # ---- queryT [e', b] ----------------------------------------
            qT_sb = singles.tile([128, EC, BS], f32)
            for m in range(EC):
                pq = ppre.tile([128, BS], f32, tag="pre")
                nc.tensor.transpose(pq, query_sb[:, m * 128 : (m + 1) * 128], ident[0:BS, 0:BS])
                nc.vector.tensor_copy(qT_sb[:, m, :], pq)

            # ---- qbias[e] = bq @ Wk  (row [1, E]) ----------------------
            qbias_sb = singles.tile([1, E], f32)
            pqb = ppre.tile([1, E], f32, tag="pre")
            for c in range(EC):
                nc.tensor.matmul(
                    pqb, bqT[:, c : c + 1], wk_sb[:, c, :],
                    start=(c == 0), stop=(c == EC - 1),
                )
            nc.scalar.copy(qbias_sb, pqb)

            # ---- qp[b, e] = query @ M  [BS, E] -------------------------
            qp_sb = singles.tile([BS, E], f32)
            pqp = ppre.tile([BS, E], f32, tag="pre")
            for m in range(EC):
                nc.tensor.matmul(
                    pqp, qT_sb[:, m, :], m_sb[:, m, :],
                    start=(m == 0), stop=(m == EC - 1),
                )
            nc.scalar.copy(qp_sb, pqp)

            # ---- v[e'] = Wq.T @ bk;  qb_raw[b] = query @ v -------------
            v_sb = singles.tile([128, EC], f32)
            for m in range(EC):
                pv = ppre.tile([128, 1], f32, tag="pre")
                for c in range(EC):
                    nc.tensor.matmul(
                        pv, wq_sb[:, c, m * 128 : (m + 1) * 128], bkT[:, c : c + 1],
                        start=(c == 0), stop=(c == EC - 1),
                    )
                nc.vector.tensor_copy(v_sb[:, m : m + 1], pv)
            qb_sb = singles.tile([BS, 1], f32)
            pqbv = ppre.tile([BS, 1], f32, tag="pre")
            for m in range(EC):
                nc.tensor.matmul(
                    pqbv, qT_sb[:, m, :], v_sb[:, m : m + 1],
                    start=(m == 0), stop=(m == EC - 1),
                )
            nc.vector.tensor_copy(qb_sb, pqbv)

            # ---- dot(bq, bk) scalar ------------------------------------
            trash4 = singles.tile([128, EC], f32)
            dotp = singles.tile([128, 1], f32)
            nc.vector.tensor_mul(trash4, bqT, bkT)
            nc.vector.tensor_reduce(
                dotp, trash4, axis=mybir.AxisListType.X, op=Alu.add
            )
            pdot = ppre.tile([1, 1], f32, tag="pre")
            nc.tensor.matmul(pdot, dotp, ones_col, start=True, stop=True)
            dot_sb = singles.tile([1, 1], f32)
            nc.vector.tensor_copy(dot_sb, pdot)

            # ---- qb row [1, BS] = qb_raw.T + dot(bq,bk) ----------------
            pqbrow = ppre.tile([1, BS], f32, tag="pre")
            nc.tensor.transpose(pqbrow, qb_sb, ident[0:BS, 0:BS])
            qbrow_sb = singles.tile([1, BS], f32)
            nc.scalar.activation(
                qbrow_sb, pqbrow, Act.Identity, bias=dot_sb[0:1, 0:1], scale=1.0
            )
            # broadcast to all partitions: qbb[p, b] = qb[b]
            pqbb = ppre.tile([128, BS], f32, tag="pre")
            nc.tensor.matmul(pqbb, ones_row, qbrow_sb, start=True, stop=True)
            qbb_sb = singles.tile([128, BS], f32)
            nc.vector.tensor_copy(qbb_sb, pqbb)

            # ---- mask -> m01T [p, b, k]  (1.0 = keep, 0.0 = masked) ----
            maskf = singles.tile([BS, S], f32)
            nc.vector.tensor_copy(maskf, mask_sb)  # int32 -> f32 convert
            m01 = singles.tile([BS, S], f32)
            nc.vector.tensor_scalar(
                out=m01, in0=maskf, scalar1=-1.0, scalar2=1.0,
                op0=Alu.mult, op1=Alu.add,
            )
            m01T = singles.tile([128, BS, KC], f32)
            for k in range(KC):
                pmt = pepi.tile([128, BS], f32, tag="epi")
                nc.tensor.transpose(pmt, m01[:, k * 128 : (k + 1) * 128], ident[0:BS, 0:BS])
                nc.vector.tensor_copy(m01T[:, :, k], pmt)

            # ---- main loop: scores[p, b, k] ----------------------------
            # Pipeline in half-batch units (2 MB DMA / [128, 8, 512] tiles).
            # One wide multiply per unit amortizes DVE per-op overhead; the
            # e-reduction is split between VectorE (wide innermost-dim
            # tensor_reduce) and ScalarE (per-chunk activation accum) so
            # both engines sit just under the HBM-stream floor.
            scores = singles.tile([128, BS, KC], f32)
            HK = KC // 2  # 8 s-chunks per half-batch unit
            n_units = BS * 2
            # ~9 of 32 units reduced on DVE, spread evenly
            dve_red = {u for u in range(n_units) if u % 4 == 2 and u // 4 < 9}
            dve_red |= {3, 17}
            target_units = target_d.rearrange("b (h k p) e -> (b h) p k e", h=2, p=128)
            pb_cur = None
            for u in range(n_units):
                b, h = divmod(u, 2)
                if h == 0:
                    pb_cur = pqpb.tile([128, E], f32, tag="qpb")
                    nc.tensor.matmul(
                        pb_cur, sel_sb[:, b, :], qp_sb, start=True, stop=False
                    )
                    nc.tensor.matmul(pb_cur, ones_row, qbias_sb, start=False, stop=True)
                tgt = tgtp.tile([128, HK, E], f32, tag="tgt")
                nc.sync.dma_start(out=tgt, in_=target_units[u])
                prod = prodp.tile([128, HK, E], f32, tag="prod")
                pb_b = bass.AP(
                    tensor=pb_cur.tensor, offset=pb_cur.offset,
                    ap=[pb_cur.ap[0], [0, HK], pb_cur.ap[1]],
                )
                nc.vector.tensor_mul(prod, tgt, pb_b)
                if u in dve_red:
                    nc.vector.tensor_reduce(
                        scores[:, b, h * HK : (h + 1) * HK], prod,
                        axis=mybir.AxisListType.X, op=Alu.add,
                    )
                else:
                    for k in range(HK):
                        tr = trashp.tile([128, E], mybir.dt.bfloat16, tag="trash")
                        nc.scalar.activation(
                            tr, prod[:, k, :], Act.Copy,
                            accum_out=scores[:, b, h * HK + k : h * HK + k + 1],
                        )

            # ---- softmax epilogue (s on partitions) --------------------
            # scores += qb[b]  (ACT accum_out cannot carry an initial value)
            scores2 = singles.tile([128, BS, KC], f32)
            for b in range(BS):
                nc.vector.tensor_scalar(
                    out=scores2[:, b, :], in0=scores[:, b, :],
                    scalar1=qbb_sb[:, b : b + 1], scalar2=None, op0=Alu.add,
                )
            t_sb = singles.tile([128, BS, KC], f32)
            nc.scalar.activation(t_sb, scores2, Act.Tanh)
            e_sb = singles.tile([128, BS, KC], f32)
            nc.scalar.activation(e_sb, t_sb, Act.Exp, scale=10.0)
            e2_sb = singles.tile([128, BS, KC], f32)
            nc.vector.tensor_mul(e2_sb, e_sb, m01T)

            part_sb = singles.tile([128, BS], f32)
            for b in range(BS):
                nc.vector.tensor_reduce(
                    part_sb[:, b : b + 1], e2_sb[:, b, :],
                    axis=mybir.AxisListType.X, op=Alu.add,
                )
            pden = pepi.tile([1, BS], f32, tag="epi")
            nc.tensor.matmul(pden, ones_col, part_sb, start=True, stop=True)
            recip_sb = singles.tile([1, BS], f32)
            nc.vector.reciprocal(recip_sb, pden)
            prb = pepi.tile([128, BS], f32, tag="epi")
            nc.tensor.matmul(prb, ones_row, recip_sb, start=True, stop=True)
            rb_sb = singles.tile([128, BS], f32)
            nc.vector.tensor_copy(rb_sb, prb)

            a_sb = singles.tile([128, BS, KC], f32)
            for b in range(BS):
                nc.vector.tensor_scalar(
                    out=a_sb[:, b, :], in0=e2_sb[:, b, :],
                    scalar1=rb_sb[:, b : b + 1], scalar2=None, op0=Alu.mult,
                )

            # ---- transpose to row-major [b*KC + k, p] and store --------
            alpha_flat = alpha_d.rearrange("b (k p) -> (b k) p", p=128)
            for h in range(2):
                pat = pepi.tile([128, 128], f32, tag="epi")
                nc.tensor.transpose(
                    pat, a_sb[:, h * (BS // 2) : (h + 1) * (BS // 2), :], ident
                )
                at_sb = singles.tile([128, 128], f32, tag=f"at{h}")
                nc.vector.tensor_copy(at_sb, pat)
                nc.sync.dma_start(
                    out=alpha_flat[h * 128 : (h + 1) * 128, :], in_=at_sb
                )

    _legalize_sync_waits(nc)
    return nc


_NC_CACHE = None


def kernel(query, target, mask, Wq, bq, Wk, bk):
    global _NC_CACHE
    _install_axon_profile_shim()
    from concourse.bass_utils import run_bass_kernel_spmd

    query = np.ascontiguousarray(np.asarray(query, dtype=np.float32))
    target = np.ascontiguousarray(np.asarray(target, dtype=np.float32))
    mask = np.ascontiguousarray(np.asarray(mask, dtype=np.int32))
    Wq = np.ascontiguousarray(np.asarray(Wq, dtype=np.float32))
    bq = np.ascontiguousarray(np.asarray(bq, dtype=np.float32))
    Wk = np.ascontiguousarray(np.asarray(Wk, dtype=np.float32))
    bk = np.ascontiguousarray(np.asarray(bk, dtype=np.float32))

    if _NC_CACHE is None:
        _NC_CACHE = build_kernel()
    nc = _NC_CACHE

    in_maps = []
    for i in range(NCORES):
        sl = slice(i * BS, (i + 1) * BS)
        in_maps.append(
            {
                "query": query[sl],
                "target": target[sl],
                "mask": mask[sl],
                "Wq": Wq,
                "bq": bq,
                "Wk": Wk,
                "bk": bk,
            }
        )

    res = run_bass_kernel_spmd(nc, in_maps, list(range(NCORES)))
    out = np.concatenate([res.results[i]["alpha"] for i in range(NCORES)], axis=0)
    return out.astype(np.float32)
